# revision 6
# baseline (speedup 1.0000x reference)
"""Trainium2 Bass kernel for nn_Conv_LI — v7: packed-pair conv, lean DVE queue.

Same math as v5 (conv-first via linearity, then out[t] = 0.8*out[t-1] +
y[t-1] on DVE). The conv drops from 5 to 4 PE streams per timestep:

  M1 streams x at offset +1 with a 128-column stationary [b1 | b0]:
     psum rows 0-63   (lo) += b1^T x[:, c+1]  -> aligned for out col c
     psum rows 64-127 (hi) += b0^T x[:, c+1]  -> out needs hi[c-1]
  M2..M4 accumulate b2, b3, b4 into the SAME psum lo rows at their aligned
     offsets (+2, +3, +4), start=False.

Fixup per timestep:
  ScalarE: t_hi[:, 1:513] = copy(psum hi rows)   (col 0 stays zero: the
           dropped hi[-1] term is b0^T * the zero w-halo column, exactly 0)
  DVE:     u = psum lo + t_hi[:, 0:512]          (one add, same partitions)
           out = 0.8*prev + u                    (recurrence STT)

Engine budget/ts: PE 4x427=1708 ns (bottleneck), DVE ~1.3 us? no: 2 ops
~1.3 us/ts? DVE 690+632=1.32 us < PE?? PE 1708 > 1322 ✓, Act ~580, DMA ~380.
"""

import numpy as np

T_FULL, H_FULL, W_FULL = 256, 512, 512
N_CORES = 8
HC = H_FULL // N_CORES  # 64 output rows per core
HP = HC + 4             # 68 partition rows incl 2+2 halo
WP = W_FULL + 4         # 516 padded width
TW = 16                 # timesteps per SBUF input window
TC = 4                  # timesteps per input DMA chunk
DECAY = 0.8

_PROG_CACHE = {}


def _build_program(t_total):
    import concourse.bacc as bacc
    import concourse.mybir as mybir
    import concourse.tile as tile

    f16 = mybir.dt.float16
    f32 = mybir.dt.float32
    mult = mybir.AluOpType.mult
    add = mybir.AluOpType.add

    assert t_total % TW == 0
    nwin = t_total // TW

    nc = bacc.Bacc(None, target_bir_lowering=False)
    x = nc.dram_tensor("x", [t_total, HP, WP], f16, kind="ExternalInput")
    # lw layout: [b1 | b0 | b2 | b3 | b4], each [HP, HC]
    lw_d = nc.dram_tensor("lw", [HP, 5 * HC], f16, kind="ExternalInput")
    out = nc.dram_tensor("out", [t_total, HC, W_FULL], f16, kind="ExternalOutput")

    with tile.TileContext(nc) as tc:
        with (
            tc.tile_pool(name="const", bufs=1) as cpool,
            tc.tile_pool(name="xw", bufs=4) as xpool,
            tc.tile_pool(name="ob", bufs=4) as opool,
            tc.tile_pool(name="uu", bufs=3) as upool,
            tc.tile_pool(name="ps", bufs=8, space="PSUM") as ppool,
        ):
            lw = cpool.tile([HP, 5 * HC], f16)
            nc.sync.dma_start(out=lw[:HP, :], in_=lw_d[:, :])
            zt = cpool.tile([HC, W_FULL], f16)
            nc.vector.memset(zt[:HC, :], 0.0)
            # static shifted-hi staging tiles; col 0 zeroed once and never
            # written again (Act copy only touches cols 1:513)
            ths = []
            for i in range(3):
                t = cpool.tile([HC, W_FULL + 1], f32, name=f"th{i}")
                nc.vector.memset(t[:HC, :], 0.0)
                ths.append(t)

            prev = None
            for win in range(nwin):
                t0 = win * TW
                xw = xpool.tile([HP, TW * WP], f16)
                for c0 in range(0, TW, TC):
                    nc.sync.dma_start(
                        out=xw[:HP, c0 * WP : (c0 + TC) * WP].rearrange(
                            "h (t w) -> h t w", t=TC
                        ),
                        in_=x[t0 + c0 : t0 + c0 + TC].rearrange("t h w -> h t w"),
                    )
                for pr in range(TW // 4):
                    sa = 4 * pr
                    ob = opool.tile([HC, 4 * W_FULL], f16)
                    for half in range(4):
                        s = sa + half
                        ps = ppool.tile([2 * HC, W_FULL], f32)
                        # M1: packed pair [b1 | b0], stream offset +1
                        nc.tensor.matmul(
                            ps[0 : 2 * HC, :],
                            lw[:HP, 0 : 2 * HC],
                            xw[:HP, s * WP + 1 : s * WP + 1 + W_FULL],
                            start=True,
                            stop=False,
                            skip_group_check=True,
                        )
                        # M2..M4: singles b2, b3, b4 into lo rows, aligned
                        for k in range(3):
                            dx = 2 + k
                            nc.tensor.matmul(
                                ps[0:HC, :],
                                lw[:HP, (2 + k) * HC : (3 + k) * HC],
                                xw[:HP, s * WP + dx : s * WP + dx + W_FULL],
                                start=False,
                                stop=(k == 2),
                                skip_group_check=True,
                            )
                        # ScalarE: stage shifted hi half (col 0 stays zero)
                        th = ths[s % 3]
                        nc.scalar.copy(
                            out=th[0:HC, 1 : W_FULL + 1],
                            in_=ps[HC : 2 * HC, 0:W_FULL],
                        )
                        # DVE: u = lo + shifted hi, then recurrence
                        u = upool.tile([HC, W_FULL], f32)
                        nc.vector.tensor_tensor(
                            out=u[0:HC, :],
                            in0=ps[0:HC, :],
                            in1=th[0:HC, 0:W_FULL],
                            op=add,
                        )
                        cur = ob[0:HC, half * W_FULL : (half + 1) * W_FULL]
                        p = zt[:HC, :] if prev is None else prev
                        nc.vector.scalar_tensor_tensor(
                            out=cur,
                            in0=p,
                            scalar=DECAY,
                            in1=u[0:HC, :],
                            op0=mult,
                            op1=add,
                        )
                        prev = cur
                    nc.gpsimd.dma_start(
                        out=out[t0 + sa : t0 + sa + 4].rearrange("t h w -> h t w"),
                        in_=ob[0:HC, :].rearrange("h (t w) -> h t w", t=4),
                    )
    nc.finalize()
    return nc


def _get_program(t_total):
    if t_total not in _PROG_CACHE:
        _PROG_CACHE[t_total] = _build_program(t_total)
    return _PROG_CACHE[t_total]


def _host_prep(x, k, t_total):
    """Build per-core shifted+padded fp16 inputs and banded lhsT matrices."""
    x = np.asarray(x, dtype=np.float32)
    k = np.asarray(k, dtype=np.float32)
    # time-shift by one (out[t] uses y[t-1]) and zero-pad h/w by 2
    xs = np.zeros((t_total, H_FULL + 4, W_FULL + 4), np.float16)
    xs[1:, 2 : H_FULL + 2, 2 : W_FULL + 2] = x[: t_total - 1, 0].astype(np.float16)
    # banded conv matrices: band[dx][p, j] = k[p - j, dx] for p - j in [0, 5)
    bands = np.zeros((5, HP, HC), np.float16)
    j = np.arange(HC)
    for dy in range(5):
        for dx in range(5):
            bands[dx, j + dy, j] = k[dy, dx]
    # v6 layout: [b1 | b0 | b2 | b3 | b4]
    order = [1, 0, 2, 3, 4]
    lwh = np.concatenate([bands[i] for i in order], axis=1)
    lwh = np.ascontiguousarray(lwh)
    in_maps = []
    for c in range(N_CORES):
        xc = np.ascontiguousarray(xs[:, c * HC : c * HC + HP, :])
        in_maps.append({"x": xc, "lw": lwh})
    return in_maps


def kernel(x, kernel):
    from concourse.bass_utils import run_bass_kernel_spmd

    t_total = x.shape[0]
    in_maps = _host_prep(x, kernel, t_total)
    nc = _get_program(t_total)
    res = run_bass_kernel_spmd(nc, in_maps, list(range(N_CORES)))
    out = np.empty((t_total, 1, H_FULL, W_FULL), np.float32)
    for c in range(N_CORES):
        out[:, 0, c * HC : (c + 1) * HC, :] = np.asarray(
            res.results[c]["out"], dtype=np.float32
        )
    return out


# revision 8
# speedup vs baseline: 1.2042x; 1.2042x over previous
"""Trainium2 Bass kernel for nn_Conv_LI — v8: packed-pair conv, paired M1 issue.

Same math as v5 (conv-first via linearity, then out[t] = 0.8*out[t-1] +
y[t-1] on DVE). The conv drops from 5 to 4 PE streams per timestep:

  M1 streams x at offset +1 with a 128-column stationary [b1 | b0]:
     psum rows 0-63   (lo) += b1^T x[:, c+1]  -> aligned for out col c
     psum rows 64-127 (hi) += b0^T x[:, c+1]  -> out needs hi[c-1]
  M2..M4 accumulate b2, b3, b4 into the SAME psum lo rows at their aligned
     offsets (+2, +3, +4), start=False.

Fixup per timestep:
  ScalarE: t_hi[:, 1:513] = copy(psum hi rows)   (col 0 stays zero: the
           dropped hi[-1] term is b0^T * the zero w-halo column, exactly 0)
  DVE:     u = psum lo + t_hi[:, 0:512]          (one add, same partitions)
           out = 0.8*prev + u                    (recurrence STT)

Engine budget/ts: PE 4x427=1708 ns (bottleneck), DVE ~1.3 us? no: 2 ops
~1.3 us/ts? DVE 690+632=1.32 us < PE?? PE 1708 > 1322 ✓, Act ~580, DMA ~380.
"""

import numpy as np

T_FULL, H_FULL, W_FULL = 256, 512, 512
N_CORES = 8
HC = H_FULL // N_CORES  # 64 output rows per core
HP = HC + 4             # 68 partition rows incl 2+2 halo
WP = W_FULL + 4         # 516 padded width
TW = 16                 # timesteps per SBUF input window
TC = 4                  # timesteps per input DMA chunk
DECAY = 0.8

_PROG_CACHE = {}


def _build_program(t_total):
    import concourse.bacc as bacc
    import concourse.mybir as mybir
    import concourse.tile as tile

    f16 = mybir.dt.float16
    f32 = mybir.dt.float32
    mult = mybir.AluOpType.mult
    add = mybir.AluOpType.add

    assert t_total % TW == 0
    nwin = t_total // TW

    nc = bacc.Bacc(None, target_bir_lowering=False)
    x = nc.dram_tensor("x", [t_total, HP, WP], f16, kind="ExternalInput")
    # lw layout: [b1 | b0 | b2 | b3 | b4], each [HP, HC]
    lw_d = nc.dram_tensor("lw", [HP, 5 * HC], f16, kind="ExternalInput")
    out = nc.dram_tensor("out", [t_total, HC, W_FULL], f16, kind="ExternalOutput")

    with tile.TileContext(nc) as tc:
        with (
            tc.tile_pool(name="const", bufs=1) as cpool,
            tc.tile_pool(name="xw", bufs=4) as xpool,
            tc.tile_pool(name="ob", bufs=4) as opool,
            tc.tile_pool(name="th", bufs=3) as thpool,
            tc.tile_pool(name="uu", bufs=3) as upool,
            tc.tile_pool(name="ps", bufs=8, space="PSUM") as ppool,
        ):
            lw = cpool.tile([HP, 5 * HC], f16)
            nc.sync.dma_start(out=lw[:HP, :], in_=lw_d[:, :])
            zt = cpool.tile([HC, W_FULL], f16)
            nc.vector.memset(zt[:HC, :], 0.0)

            prev = None
            for win in range(nwin):
                t0 = win * TW
                xw = xpool.tile([HP, TW * WP], f16)
                for c0 in range(0, TW, TC):
                    nc.sync.dma_start(
                        out=xw[:HP, c0 * WP : (c0 + TC) * WP].rearrange(
                            "h (t w) -> h t w", t=TC
                        ),
                        in_=x[t0 + c0 : t0 + c0 + TC].rearrange("t h w -> h t w"),
                    )
                for pr in range(TW // 2):
                    sa = 2 * pr
                    ob = opool.tile([HC, 2 * W_FULL], f16)
                    # both packed M1s back-to-back so the wide LDWEIGHTS
                    # hides under the other M1's longer stream
                    pss = []
                    for half in range(2):
                        s = sa + half
                        ps = ppool.tile([2 * HC, W_FULL], f32)
                        pss.append(ps)
                        nc.tensor.matmul(
                            ps[0 : 2 * HC, :],
                            lw[:HP, 0 : 2 * HC],
                            xw[:HP, s * WP + 1 : s * WP + 1 + W_FULL],
                            start=True,
                            stop=False,
                            skip_group_check=True,
                        )
                    # then the six singles (64-col LDWEIGHTS hide easily)
                    for half in range(2):
                        s = sa + half
                        ps = pss[half]
                        for k in range(3):
                            dx = 2 + k
                            nc.tensor.matmul(
                                ps[0:HC, :],
                                lw[:HP, (2 + k) * HC : (3 + k) * HC],
                                xw[:HP, s * WP + dx : s * WP + dx + W_FULL],
                                start=False,
                                stop=(k == 2),
                                skip_group_check=True,
                            )
                    for half in range(2):
                        s = sa + half
                        ps = pss[half]
                        # ScalarE: stage shifted hi half (col 0 stays zero)
                        th = thpool.tile([HC, W_FULL + 1], f32)
                        nc.vector.memset(th[:HC, 0:1], 0.0)
                        nc.scalar.copy(
                            out=th[0:HC, 1 : W_FULL + 1],
                            in_=ps[HC : 2 * HC, 0:W_FULL],
                        )
                        # DVE: u = lo + shifted hi, then recurrence
                        u = upool.tile([HC, W_FULL], f32)
                        nc.vector.tensor_tensor(
                            out=u[0:HC, :],
                            in0=ps[0:HC, :],
                            in1=th[0:HC, 0:W_FULL],
                            op=add,
                        )
                        cur = ob[0:HC, half * W_FULL : (half + 1) * W_FULL]
                        p = zt[:HC, :] if prev is None else prev
                        nc.vector.scalar_tensor_tensor(
                            out=cur,
                            in0=p,
                            scalar=DECAY,
                            in1=u[0:HC, :],
                            op0=mult,
                            op1=add,
                        )
                        prev = cur
                    nc.gpsimd.dma_start(
                        out=out[t0 + sa : t0 + sa + 2].rearrange("t h w -> h t w"),
                        in_=ob[0:HC, :].rearrange("h (t w) -> h t w", t=2),
                    )
    nc.finalize()
    return nc


def _get_program(t_total):
    if t_total not in _PROG_CACHE:
        _PROG_CACHE[t_total] = _build_program(t_total)
    return _PROG_CACHE[t_total]


def _host_prep(x, k, t_total):
    """Build per-core shifted+padded fp16 inputs and banded lhsT matrices."""
    x = np.asarray(x, dtype=np.float32)
    k = np.asarray(k, dtype=np.float32)
    # time-shift by one (out[t] uses y[t-1]) and zero-pad h/w by 2
    xs = np.zeros((t_total, H_FULL + 4, W_FULL + 4), np.float16)
    xs[1:, 2 : H_FULL + 2, 2 : W_FULL + 2] = x[: t_total - 1, 0].astype(np.float16)
    # banded conv matrices: band[dx][p, j] = k[p - j, dx] for p - j in [0, 5)
    bands = np.zeros((5, HP, HC), np.float16)
    j = np.arange(HC)
    for dy in range(5):
        for dx in range(5):
            bands[dx, j + dy, j] = k[dy, dx]
    # v6 layout: [b1 | b0 | b2 | b3 | b4]
    order = [1, 0, 2, 3, 4]
    lwh = np.concatenate([bands[i] for i in order], axis=1)
    lwh = np.ascontiguousarray(lwh)
    in_maps = []
    for c in range(N_CORES):
        xc = np.ascontiguousarray(xs[:, c * HC : c * HC + HP, :])
        in_maps.append({"x": xc, "lw": lwh})
    return in_maps


def kernel(x, kernel):
    from concourse.bass_utils import run_bass_kernel_spmd

    t_total = x.shape[0]
    in_maps = _host_prep(x, kernel, t_total)
    nc = _get_program(t_total)
    res = run_bass_kernel_spmd(nc, in_maps, list(range(N_CORES)))
    out = np.empty((t_total, 1, H_FULL, W_FULL), np.float32)
    for c in range(N_CORES):
        out[:, 0, c * HC : (c + 1) * HC, :] = np.asarray(
            res.results[c]["out"], dtype=np.float32
        )
    return out
